# revision 31
# baseline (speedup 1.0000x reference)
"""Trainium2 Bass kernel for nn_Attention_85074712199827.

Computes, for hidden [1,32,1024], encoder_outputs [32,2048,1024],
W_attn [1024,2048], b_attn [1024], v [1024]:

    h_proj  = hidden[0] @ W_attn[:, :1024].T
    e_proj  = encoder_outputs @ W_attn[:, 1024:].T
    energy  = tanh(e_proj + h_proj[:, None, :] + b_attn)
    att     = energy @ v
    out     = softmax(att, axis=1)          # [32, 2048] float32

Distribution: data-parallel over the batch across 8 NeuronCores (4
batch rows per core); the tiny parameters are replicated. Host-side
prep casts to bf16 and lays encoder_outputs out pre-transposed in the
exact per-partition SBUF layout, so the device pipeline is a single
stream of matmul chains (PE is the bottleneck at ~221ns per
[128c,512f] matmul).

Softmax skips the max-subtraction: att logits are O(3) for this
problem, exp stays comfortably inside f32 range, and softmax is
shift-invariant so the result matches the reference.

Self-contained: only environment packages (concourse, numpy, ml_dtypes)
are imported; all shapes/sharding are hardcoded for this problem.
"""

from contextlib import ExitStack

import ml_dtypes
import numpy as np

import concourse.bass as bass
import concourse.tile as tile
from concourse import bacc, mybir

F32 = mybir.dt.float32
BF16 = mybir.dt.bfloat16
AF = mybir.ActivationFunctionType
P = 128


def build_nc(b_loc=4, s=2048, h=1024, n_cores=8, sb=512,
             warmup_mm=10, inp_bufs=5, pe_bufs=5, en_bufs=4,
             hproj_after=1):
    assert hproj_after <= 1, "h_proj must precede the first v-dot (deadlock)"
    n_sb = s // sb          # s-blocks per batch row
    n_hc = h // P           # contraction chunks
    n_ot = h // P           # output (o) tiles
    n_units = b_loc * n_sb

    nc = bacc.Bacc("TRN2", target_bir_lowering=False, debug=False,
                   num_devices=n_cores)

    wt = nc.dram_tensor("wt", [2 * h, h], BF16, kind="ExternalInput").ap()
    hiddenT = nc.dram_tensor("hiddenT", [h, b_loc], BF16, kind="ExternalInput").ap()
    b_attn = nc.dram_tensor("b_attn", [h], F32, kind="ExternalInput").ap()
    v = nc.dram_tensor("v", [h], F32, kind="ExternalInput").ap()
    # host-pre-transposed: encT[p, ((b*n_sb+isb)*n_hc+hc)*sb + s'] =
    #   enc[b, isb*sb+s', hc*128+p]
    encT = nc.dram_tensor("encT", [P, n_units * n_hc * sb], BF16,
                          kind="ExternalInput").ap()
    out = nc.dram_tensor("out", [b_loc, s], F32, kind="ExternalOutput").ap()

    with tile.TileContext(nc) as tc, ExitStack() as ctx:
        const = ctx.enter_context(tc.tile_pool(name="const", bufs=1))
        psmall = ctx.enter_context(tc.tile_pool(name="psmall", bufs=1, space="PSUM"))
        inp = ctx.enter_context(tc.tile_pool(name="inp", bufs=inp_bufs))
        en_p = ctx.enter_context(tc.tile_pool(name="energy", bufs=en_bufs))
        acc_p = ctx.enter_context(tc.tile_pool(name="acc", bufs=2))
        tmp_p = ctx.enter_context(tc.tile_pool(name="tmp", bufs=3))
        pe_p = ctx.enter_context(tc.tile_pool(name="psum_e", bufs=pe_bufs, space="PSUM"))
        pa_p = ctx.enter_context(tc.tile_pool(name="psum_att", bufs=2, space="PSUM"))

        wt_r = wt.rearrange("(jc p) o -> p jc o", p=P)
        wt_bf = const.tile([P, 2 * n_hc, h], BF16)

        # ---- DMA priority order: unit0 tile, then We o-chunks (chain
        # ot_k unblocks as its chunk lands), smalls, Wh late (h_proj is
        # emitted a few chains in) ----
        blk0 = 3 * n_sb          # first processed unit is (b=3, isb=0)
        et0 = inp.tile([P, n_hc * sb], BF16, name="it")
        nc.sync.dma_start(
            et0[:], encT[:, blk0 * n_hc * sb:(blk0 + 1) * n_hc * sb])

        for ot in range(n_ot):
            nc.sync.dma_start(
                wt_bf[:, n_hc:2 * n_hc, ot * P:(ot + 1) * P],
                wt_r[:, n_hc:2 * n_hc, ot * P:(ot + 1) * P])

        hT_bf = const.tile([P, n_hc, b_loc], BF16)
        nc.scalar.dma_start(hT_bf[:], hiddenT.rearrange("(hc p) b -> p hc b", p=P))
        baT = const.tile([P, n_ot], F32)
        nc.scalar.dma_start(baT[:], b_attn.rearrange("(oc p) -> p oc", p=P))
        vabsT = const.tile([P, n_ot], F32)
        nc.scalar.dma_start(vabsT[:], v.rearrange("(oc p) -> p oc", p=P))

        # et1 before Wh: unit1's tile is needed (~26us) sooner than
        # h_proj consumes Wh (~15us after queue drain)
        et1 = inp.tile([P, n_hc * sb], BF16, name="it")
        blk1 = 3 * n_sb + 1
        nc.sync.dma_start(
            et1[:], encT[:, blk1 * n_hc * sb:(blk1 + 1) * n_hc * sb])

        for c in range(2):
            q = n_hc // 2
            nc.sync.dma_start(
                wt_bf[:, c * q:(c + 1) * q, :],
                wt_r[:, c * q:(c + 1) * q, :])

        # ---- PE warmup: dependency-free matmuls ramp the clock while
        # the first tiles stream in ----
        wz = const.tile([P, sb], BF16)
        nc.vector.memset(wz[:], 0)
        for _ in range(warmup_mm):
            pw = psmall.tile([P, sb], F32, name="pw", tag="ps")
            nc.tensor.matmul(pw[:], wz[:, :P], wz[:], start=True, stop=True)

        ones = const.tile([P, 1], BF16)
        nc.vector.memset(ones[:], 1)

        # exp results + per-block sums; batch rows 0..2 live on psum/SBUF
        # partition 32*b (matmul out base partition must be 0/32/64);
        # b=3 shares partition 0 using the second s-wide column half and
        # is processed FIRST so its normalize hides under later units.
        # Unwritten partitions hold garbage; the epilogue math on them is
        # never DMA'd out, so no memset is needed.
        e_rows = const.tile([P, 2 * s], F32)
        esum = const.tile([P, 2 * n_sb], F32)
        nc.gpsimd.memset(esum[:], 0)

        hb = const.tile([P, n_ot, b_loc], F32, name="hb")

        def emit_hproj():
            for ot in range(n_ot):
                ph = psmall.tile([P, b_loc], F32, name="ph", tag="ps")
                for hc in range(n_hc):
                    nc.tensor.matmul(
                        ph[:], wt_bf[:, hc, ot * P:(ot + 1) * P], hT_bf[:, hc, :],
                        start=(hc == 0), stop=(hc == n_hc - 1))
                nc.vector.tensor_tensor(
                    hb[:, ot, :], ph[:],
                    baT[:, ot, None].to_broadcast((P, b_loc)),
                    mybir.AluOpType.add)

        o_rows = const.tile([P, 2 * s], F32)

        def unit_slots(b, isb):
            if b == 3:
                return 0, s + isb * sb, n_sb + isb
            return 32 * b, isb * sb, isb

        def process_unit(first, et, b, isb, emit_deferred, late_dmas=()):
            row, col, esc = unit_slots(b, isb)
            # First unit: tanh emission lags until after emit_hproj() —
            # a tanh emitted before hb's writers exist would read stale
            # SBUF (program-order dep tracking).
            tanh_lag = hproj_after + 1 if first else 0
            pes = []
            acc = acc_p.tile([P, sb], F32, name="acc")
            # intermediate accumulation in f32; the LAST add rounds once
            # to bf16 so the partition-reduce matmul runs at bf16 rate
            acc_bf = acc_p.tile([P, sb], BF16, name="accb")

            def flush_tanh():
                pot, pe = pes.pop(0)
                eng = en_p.tile([P, sb], BF16, name="eng")
                nc.scalar.activation(
                    eng[:], pe[:], AF.Tanh, bias=hb[:, pot, b:b + 1])
                # |v|-weighted accumulate across the 8 energy tiles on
                # Vector (sign(v) is folded into We/hb host-side); the
                # partition reduction then costs ONE matmul per unit
                if pot == 0:
                    nc.vector.tensor_scalar_mul(
                        acc[:], eng[:], vabsT[:, 0:1])
                else:
                    tmp = tmp_p.tile([P, sb], F32, name="tmp")
                    nc.vector.tensor_scalar_mul(
                        tmp[:], eng[:], vabsT[:, pot:pot + 1])
                    nc.vector.tensor_tensor(
                        acc_bf[:] if pot == n_ot - 1 else acc[:],
                        acc[:], tmp[:], mybir.AluOpType.add)

            for ot in range(n_ot):
                pe = pe_p.tile([P, sb], F32, name="pe")
                for hc in range(n_hc):
                    nc.tensor.matmul(
                        pe[:], wt_bf[:, n_hc + hc, ot * P:(ot + 1) * P],
                        et[:, hc * sb:(hc + 1) * sb],
                        start=(hc == 0), stop=(hc == n_hc - 1))
                pes.append((ot, pe))
                for dot, dtile, dsrc in late_dmas:
                    if ot == dot:
                        # issued on the in-order Scalar queue behind the
                        # tanhs: delays the transfer until the critical
                        # startup DMAs (et0/et1/weights) have drained —
                        # queues round-robin, so an early issue would
                        # steal bandwidth from them
                        nc.scalar.dma_start(dtile[:], dsrc)
                if first and ot == hproj_after:
                    emit_hproj()
                if ot == 0 and emit_deferred is not None:
                    # previous unit's reduce+exp: deferred so the PE
                    # never waits on that unit's accumulate chain
                    emit_deferred()
                if len(pes) > tanh_lag:
                    flush_tanh()
            while pes:
                flush_tanh()

            def deferred():
                pa = pa_p.tile([P, sb], F32, name="pa")
                nc.tensor.matmul(
                    pa[row:row + 1, :], ones[:], acc_bf[:],
                    start=True, stop=True, skip_group_check=True)
                nc.scalar.activation(
                    e_rows[row:row + 1, col:col + sb], pa[row:row + 1, :],
                    AF.Exp, accum_out=esum[row:row + 1, esc:esc + 1])

            return deferred

        def normalize_b3():
            # b=3 epilogue right after its 4 units: hidden under the
            # PE stream of the remaining 12 units; runs on Scalar so the
            # final b0-2 normalize (Vector) never queues behind it
            ssum_b = const.tile([P, 1], F32)
            nc.vector.tensor_reduce(
                ssum_b[:], esum[:, n_sb:], mybir.AxisListType.X,
                mybir.AluOpType.add)
            rinv_b = const.tile([P, 1], F32)
            nc.vector.reciprocal(rinv_b[:], ssum_b[:])
            nc.scalar.activation(
                o_rows[0:1, s:], e_rows[0:1, s:], AF.Copy,
                scale=rinv_b[0:1, :])
            nc.gpsimd.dma_start(out[3:4, :], o_rows[0:1, s:])

        # b=3 first (it shares partition 0 with b=0), then b=0..2
        units = [(3, isb) for isb in range(n_sb)] + \
                [(b, isb) for b in range(3) for isb in range(n_sb)]

        def unit_src(pos):
            pb, pisb = units[pos]
            blk = pb * n_sb + pisb
            return encT[:, blk * n_hc * sb:(blk + 1) * n_hc * sb]

        ets = {0: et0, 1: et1}
        late_dmas = []
        for pos, dot in ((2, 3), (3, 5)):
            et = inp.tile([P, n_hc * sb], BF16, name="it")
            ets[pos] = et
            late_dmas.append((dot, et, unit_src(pos)))

        deferred = None
        for u, (b, isb) in enumerate(units):
            for pf in range(u + 1, min(u + inp_bufs - 1, n_units)):
                if pf not in ets:
                    et = inp.tile([P, n_hc * sb], BF16, name="it")
                    nc.sync.dma_start(et[:], unit_src(pf))
                    ets[pf] = et
            deferred = process_unit(u == 0, ets.pop(u), b, isb, deferred,
                                    late_dmas if u == 0 else ())
            if u == n_sb:
                # all four b=3 exps exist in program order by now (the
                # last one rode in as unit 4's deferred block)
                normalize_b3()
        deferred()

        # ---- softmax epilogue for b=0..2 ----
        ssum = const.tile([P, 1], F32)
        nc.vector.tensor_reduce(
            ssum[:], esum[:, :n_sb], mybir.AxisListType.X, mybir.AluOpType.add)
        rinv = const.tile([P, 1], F32)
        nc.vector.reciprocal(rinv[:], ssum[:])
        nc.vector.tensor_scalar_mul(o_rows[:, :s], e_rows[:, :s], rinv[:])
        for b in range(3):
            nc.sync.dma_start(out[b:b + 1, :], o_rows[32 * b:32 * b + 1, :s])

    nc.compile()
    return nc


def make_in_maps(hidden, encoder_outputs, W_attn, b_attn, v, n_cores=8):
    hidden = np.asarray(hidden, dtype=np.float32)
    encoder_outputs = np.asarray(encoder_outputs, dtype=np.float32)
    W_attn = np.asarray(W_attn, dtype=np.float32)
    b_attn = np.asarray(b_attn, dtype=np.float32)
    v = np.asarray(v, dtype=np.float32)

    b = encoder_outputs.shape[0]
    b_loc = b // n_cores
    s = encoder_outputs.shape[1]
    h = encoder_outputs.shape[2]
    sb = 512
    n_sb = s // sb
    n_hc = h // P
    # tanh is odd: v*tanh(E) == |v|*tanh(sign(v)*E). Fold sign(v) into
    # the weight columns and the bias so the device only scales by |v|.
    sv = np.sign(v).astype(np.float32)
    wt = np.ascontiguousarray(
        (W_attn.T * sv[None, :]).astype(ml_dtypes.bfloat16))
    b_signed = (b_attn * sv).astype(np.float32)
    v_abs = np.abs(v).astype(np.float32)
    in_maps = []
    for i in range(n_cores):
        bsl = slice(b_loc * i, b_loc * (i + 1))
        e = encoder_outputs[bsl].astype(ml_dtypes.bfloat16)
        e = e.reshape(b_loc, n_sb, sb, n_hc, P).transpose(4, 0, 1, 3, 2)
        encT = np.ascontiguousarray(e).reshape(P, b_loc * n_sb * n_hc * sb)
        in_maps.append({
            "wt": wt,
            "hiddenT": np.ascontiguousarray(
                hidden[0, bsl].T.astype(ml_dtypes.bfloat16)),
            "b_attn": b_signed,
            "v": v_abs,
            "encT": encT,
        })
    return in_maps


_NC_CACHE = {}


def _get_nc():
    if "nc" not in _NC_CACHE:
        _NC_CACHE["nc"] = build_nc(b_loc=4, s=2048, h=1024, n_cores=8)
    return _NC_CACHE["nc"]


def kernel(hidden, encoder_outputs, W_attn, b_attn, v):
    from concourse.bass_utils import run_bass_kernel_spmd

    nc = _get_nc()
    in_maps = make_in_maps(hidden, encoder_outputs, W_attn, b_attn, v,
                           n_cores=8)
    res = run_bass_kernel_spmd(nc, in_maps, core_ids=list(range(8)))
    out = np.concatenate([np.asarray(res.results[i]["out"])
                          for i in range(8)], axis=0)
    return out.astype(np.float32)


# revision 36
# speedup vs baseline: 1.0309x; 1.0309x over previous
"""Trainium2 Bass kernel for nn_Attention_85074712199827.

Computes, for hidden [1,32,1024], encoder_outputs [32,2048,1024],
W_attn [1024,2048], b_attn [1024], v [1024]:

    h_proj  = hidden[0] @ W_attn[:, :1024].T
    e_proj  = encoder_outputs @ W_attn[:, 1024:].T
    energy  = tanh(e_proj + h_proj[:, None, :] + b_attn)
    att     = energy @ v
    out     = softmax(att, axis=1)          # [32, 2048] float32

Distribution: data-parallel over the batch across 8 NeuronCores (4
batch rows per core); the tiny parameters are replicated. Host-side
prep casts to bf16 and lays encoder_outputs out pre-transposed in the
exact per-partition SBUF layout, so the device pipeline is a single
stream of matmul chains (PE is the bottleneck at ~221ns per
[128c,512f] matmul).

Softmax skips the max-subtraction: att logits are O(3) for this
problem, exp stays comfortably inside f32 range, and softmax is
shift-invariant so the result matches the reference.

Self-contained: only environment packages (concourse, numpy, ml_dtypes)
are imported; all shapes/sharding are hardcoded for this problem.
"""

from contextlib import ExitStack

import ml_dtypes
import numpy as np

import concourse.bass as bass
import concourse.tile as tile
from concourse import bacc, mybir

F32 = mybir.dt.float32
BF16 = mybir.dt.bfloat16
AF = mybir.ActivationFunctionType
P = 128


def build_nc(b_loc=4, s=2048, h=1024, n_cores=8, sb=512,
             warmup_mm=10, inp_bufs=5, pe_bufs=5, en_bufs=4,
             hproj_after=1):
    assert hproj_after <= 1, "h_proj must precede the first v-dot (deadlock)"
    n_sb = s // sb          # s-blocks per batch row
    n_hc = h // P           # contraction chunks
    n_ot = h // P           # output (o) tiles
    n_units = b_loc * n_sb

    nc = bacc.Bacc("TRN2", target_bir_lowering=False, debug=False,
                   num_devices=n_cores)

    wt = nc.dram_tensor("wt", [2 * h, h], BF16, kind="ExternalInput").ap()
    hiddenT = nc.dram_tensor("hiddenT", [h, b_loc], BF16, kind="ExternalInput").ap()
    b_attn = nc.dram_tensor("b_attn", [h], F32, kind="ExternalInput").ap()
    v = nc.dram_tensor("v", [h], F32, kind="ExternalInput").ap()
    # host-pre-transposed: encT[p, ((b*n_sb+isb)*n_hc+hc)*sb + s'] =
    #   enc[b, isb*sb+s', hc*128+p]
    encT = nc.dram_tensor("encT", [P, n_units * n_hc * sb], BF16,
                          kind="ExternalInput").ap()
    out = nc.dram_tensor("out", [b_loc, s], F32, kind="ExternalOutput").ap()

    with tile.TileContext(nc) as tc, ExitStack() as ctx:
        const = ctx.enter_context(tc.tile_pool(name="const", bufs=1))
        psmall = ctx.enter_context(tc.tile_pool(name="psmall", bufs=1, space="PSUM"))
        inp = ctx.enter_context(tc.tile_pool(name="inp", bufs=inp_bufs))
        en_p = ctx.enter_context(tc.tile_pool(name="energy", bufs=en_bufs))
        acc_p = ctx.enter_context(tc.tile_pool(name="acc", bufs=2))
        tmp_p = ctx.enter_context(tc.tile_pool(name="tmp", bufs=3))
        pe_p = ctx.enter_context(tc.tile_pool(name="psum_e", bufs=pe_bufs, space="PSUM"))
        pa_p = ctx.enter_context(tc.tile_pool(name="psum_att", bufs=2, space="PSUM"))

        wt_r = wt.rearrange("(jc p) o -> p jc o", p=P)
        wt_bf = const.tile([P, 2 * n_hc, h], BF16)

        # ---- DMA priority order: unit0 tile, then We o-chunks (chain
        # ot_k unblocks as its chunk lands), smalls, Wh late (h_proj is
        # emitted a few chains in) ----
        blk0 = 3 * n_sb          # first processed unit is (b=3, isb=0)
        et0 = inp.tile([P, n_hc * sb], BF16, name="it")
        nc.sync.dma_start(
            et0[:], encT[:, blk0 * n_hc * sb:(blk0 + 1) * n_hc * sb])

        # et1 right behind et0: queues serve outstanding transfers
        # round-robin, so issue order sets completion order
        et1 = inp.tile([P, n_hc * sb], BF16, name="it")
        blk1 = 3 * n_sb + 1
        nc.sync.dma_start(
            et1[:], encT[:, blk1 * n_hc * sb:(blk1 + 1) * n_hc * sb])

        for ot in range(n_ot):
            nc.sync.dma_start(
                wt_bf[:, n_hc:2 * n_hc, ot * P:(ot + 1) * P],
                wt_r[:, n_hc:2 * n_hc, ot * P:(ot + 1) * P])

        hT_bf = const.tile([P, n_hc, b_loc], BF16)
        nc.scalar.dma_start(hT_bf[:], hiddenT.rearrange("(hc p) b -> p hc b", p=P))
        baT = const.tile([P, n_ot], F32)
        nc.scalar.dma_start(baT[:], b_attn.rearrange("(oc p) -> p oc", p=P))
        vabsT = const.tile([P, n_ot], F32)
        nc.scalar.dma_start(vabsT[:], v.rearrange("(oc p) -> p oc", p=P))

        for c in range(2):
            q = n_hc // 2
            nc.sync.dma_start(
                wt_bf[:, c * q:(c + 1) * q, :],
                wt_r[:, c * q:(c + 1) * q, :])

        # ---- PE warmup: dependency-free matmuls ramp the clock while
        # the first tiles stream in ----
        wz = const.tile([P, sb], BF16)
        nc.vector.memset(wz[:], 0)
        for _ in range(warmup_mm):
            pw = psmall.tile([P, sb], F32, name="pw", tag="ps")
            nc.tensor.matmul(pw[:], wz[:, :P], wz[:], start=True, stop=True)

        ones = const.tile([P, 1], BF16)
        nc.vector.memset(ones[:], 1)

        # exp results + per-block sums; batch rows 0..2 live on psum/SBUF
        # partition 32*b (matmul out base partition must be 0/32/64);
        # b=3 shares partition 0 using the second s-wide column half and
        # is processed FIRST so its normalize hides under later units.
        # Unwritten partitions hold garbage; the epilogue math on them is
        # never DMA'd out, so no memset is needed.
        e_rows = const.tile([P, 2 * s], F32)
        esum = const.tile([P, 2 * n_sb], F32)
        nc.gpsimd.memset(esum[:], 0)

        hb = const.tile([P, n_ot, b_loc], F32, name="hb")

        def emit_hproj():
            for ot in range(n_ot):
                ph = psmall.tile([P, b_loc], F32, name="ph", tag="ps")
                for hc in range(n_hc):
                    nc.tensor.matmul(
                        ph[:], wt_bf[:, hc, ot * P:(ot + 1) * P], hT_bf[:, hc, :],
                        start=(hc == 0), stop=(hc == n_hc - 1))
                nc.vector.tensor_tensor(
                    hb[:, ot, :], ph[:],
                    baT[:, ot, None].to_broadcast((P, b_loc)),
                    mybir.AluOpType.add)

        o_rows = const.tile([P, 2 * s], F32)

        def unit_slots(b, isb):
            if b == 3:
                return 0, s + isb * sb, n_sb + isb
            return 32 * b, isb * sb, isb

        def process_unit(first, et, b, isb, emit_deferred):
            row, col, esc = unit_slots(b, isb)
            # First unit: tanh emission lags until after emit_hproj() —
            # a tanh emitted before hb's writers exist would read stale
            # SBUF (program-order dep tracking).
            tanh_lag = hproj_after + 1 if first else 0
            pes = []
            acc = acc_p.tile([P, sb], F32, name="acc")
            # intermediate accumulation in f32; the LAST add rounds once
            # to bf16 so the partition-reduce matmul runs at bf16 rate
            acc_bf = acc_p.tile([P, sb], BF16, name="accb")

            def flush_tanh():
                pot, pe = pes.pop(0)
                eng = en_p.tile([P, sb], BF16, name="eng")
                nc.scalar.activation(
                    eng[:], pe[:], AF.Tanh, bias=hb[:, pot, b:b + 1])
                # |v|-weighted accumulate across the 8 energy tiles on
                # Vector (sign(v) is folded into We/hb host-side); the
                # partition reduction then costs ONE matmul per unit
                if pot == 0:
                    nc.vector.tensor_scalar_mul(
                        acc[:], eng[:], vabsT[:, 0:1])
                else:
                    tmp = tmp_p.tile([P, sb], F32, name="tmp")
                    nc.vector.tensor_scalar_mul(
                        tmp[:], eng[:], vabsT[:, pot:pot + 1])
                    nc.vector.tensor_tensor(
                        acc_bf[:] if pot == n_ot - 1 else acc[:],
                        acc[:], tmp[:], mybir.AluOpType.add)

            for ot in range(n_ot):
                pe = pe_p.tile([P, sb], F32, name="pe")
                for hc in range(n_hc):
                    nc.tensor.matmul(
                        pe[:], wt_bf[:, n_hc + hc, ot * P:(ot + 1) * P],
                        et[:, hc * sb:(hc + 1) * sb],
                        start=(hc == 0), stop=(hc == n_hc - 1))
                pes.append((ot, pe))
                if first and ot == hproj_after:
                    emit_hproj()
                if ot == 0 and emit_deferred is not None:
                    # previous unit's reduce+exp: deferred so the PE
                    # never waits on that unit's accumulate chain
                    emit_deferred()
                if len(pes) > tanh_lag:
                    flush_tanh()
            while pes:
                flush_tanh()

            def deferred():
                pa = pa_p.tile([P, sb], F32, name="pa")
                nc.tensor.matmul(
                    pa[row:row + 1, :], ones[:], acc_bf[:],
                    start=True, stop=True, skip_group_check=True)
                nc.scalar.activation(
                    e_rows[row:row + 1, col:col + sb], pa[row:row + 1, :],
                    AF.Exp, accum_out=esum[row:row + 1, esc:esc + 1])

            return deferred

        def normalize_b3():
            # b=3 epilogue right after its 4 units: hidden under the
            # PE stream of the remaining 12 units; runs on Scalar so the
            # final b0-2 normalize (Vector) never queues behind it
            ssum_b = const.tile([P, 1], F32)
            nc.vector.tensor_reduce(
                ssum_b[:], esum[:, n_sb:], mybir.AxisListType.X,
                mybir.AluOpType.add)
            rinv_b = const.tile([P, 1], F32)
            nc.vector.reciprocal(rinv_b[:], ssum_b[:])
            nc.scalar.activation(
                o_rows[0:1, s:], e_rows[0:1, s:], AF.Copy,
                scale=rinv_b[0:1, :])
            nc.gpsimd.dma_start(out[3:4, :], o_rows[0:1, s:])

        # b=3 first (it shares partition 0 with b=0), then b=0..2
        units = [(3, isb) for isb in range(n_sb)] + \
                [(b, isb) for b in range(3) for isb in range(n_sb)]

        def unit_src(pos):
            pb, pisb = units[pos]
            blk = pb * n_sb + pisb
            return encT[:, blk * n_hc * sb:(blk + 1) * n_hc * sb]

        ets = {0: et0, 1: et1}
        deferred = None
        for u, (b, isb) in enumerate(units):
            for pf in range(u + 1, min(u + inp_bufs - 1, n_units)):
                if pf not in ets:
                    et = inp.tile([P, n_hc * sb], BF16, name="it")
                    nc.sync.dma_start(et[:], unit_src(pf))
                    ets[pf] = et
            deferred = process_unit(u == 0, ets.pop(u), b, isb, deferred)
            if u == n_sb:
                # all four b=3 exps exist in program order by now (the
                # last one rode in as unit 4's deferred block)
                normalize_b3()
        deferred()

        # ---- softmax epilogue for b=0..2 ----
        ssum = const.tile([P, 1], F32)
        nc.vector.tensor_reduce(
            ssum[:], esum[:, :n_sb], mybir.AxisListType.X, mybir.AluOpType.add)
        rinv = const.tile([P, 1], F32)
        nc.vector.reciprocal(rinv[:], ssum[:])
        nc.vector.tensor_scalar_mul(o_rows[:, :s], e_rows[:, :s], rinv[:])
        for b in range(3):
            nc.sync.dma_start(out[b:b + 1, :], o_rows[32 * b:32 * b + 1, :s])

    nc.compile()
    return nc


def make_in_maps(hidden, encoder_outputs, W_attn, b_attn, v, n_cores=8):
    hidden = np.asarray(hidden, dtype=np.float32)
    encoder_outputs = np.asarray(encoder_outputs, dtype=np.float32)
    W_attn = np.asarray(W_attn, dtype=np.float32)
    b_attn = np.asarray(b_attn, dtype=np.float32)
    v = np.asarray(v, dtype=np.float32)

    b = encoder_outputs.shape[0]
    b_loc = b // n_cores
    s = encoder_outputs.shape[1]
    h = encoder_outputs.shape[2]
    sb = 512
    n_sb = s // sb
    n_hc = h // P
    # tanh is odd: v*tanh(E) == |v|*tanh(sign(v)*E). Fold sign(v) into
    # the weight columns and the bias so the device only scales by |v|.
    sv = np.sign(v).astype(np.float32)
    wt = np.ascontiguousarray(
        (W_attn.T * sv[None, :]).astype(ml_dtypes.bfloat16))
    b_signed = (b_attn * sv).astype(np.float32)
    v_abs = np.abs(v).astype(np.float32)
    in_maps = []
    for i in range(n_cores):
        bsl = slice(b_loc * i, b_loc * (i + 1))
        e = encoder_outputs[bsl].astype(ml_dtypes.bfloat16)
        e = e.reshape(b_loc, n_sb, sb, n_hc, P).transpose(4, 0, 1, 3, 2)
        encT = np.ascontiguousarray(e).reshape(P, b_loc * n_sb * n_hc * sb)
        in_maps.append({
            "wt": wt,
            "hiddenT": np.ascontiguousarray(
                hidden[0, bsl].T.astype(ml_dtypes.bfloat16)),
            "b_attn": b_signed,
            "v": v_abs,
            "encT": encT,
        })
    return in_maps


_NC_CACHE = {}


def _get_nc():
    if "nc" not in _NC_CACHE:
        _NC_CACHE["nc"] = build_nc(b_loc=4, s=2048, h=1024, n_cores=8)
    return _NC_CACHE["nc"]


def kernel(hidden, encoder_outputs, W_attn, b_attn, v):
    from concourse.bass_utils import run_bass_kernel_spmd

    nc = _get_nc()
    in_maps = make_in_maps(hidden, encoder_outputs, W_attn, b_attn, v,
                           n_cores=8)
    res = run_bass_kernel_spmd(nc, in_maps, core_ids=list(range(8)))
    out = np.concatenate([np.asarray(res.results[i]["out"])
                          for i in range(8)], axis=0)
    return out.astype(np.float32)
